# revision 19
# baseline (speedup 1.0000x reference)
"""Trainium2 Bass kernel for HGNN-MLP (email/url/sender heterograph).

Math (dead-code-eliminated: out_url/out_sender unused by the return value):
  out = relu( x_email @ Wer + T @ Wcomb )[:, :] @ Wc + bc
where
  Wer  = W_email @ (Wroot_ue + Wroot_se)                      [768,128]
  T[d] = [sum x_url[src] over ue edges, deg_ue, sum x_sender[src]
          over se edges, deg_se, 1]  (12 cols, 8 replicas)
  Wcomb folds W_url@Wrel_ue, b_url@Wrel_ue, W_sender@Wrel_se,
          b_sender@Wrel_se and the bias row.

Distribution: 8-way data-parallel over destination emails (12500/core),
edge lists bucketed by dst partition on host; small weights replicated.

Device strategy per core: batched indirect-DMA gathers of source rows
(url: 8 bf16, sender: 1 bf16) followed by indirect-DMA scatter-ADD into a
DRAM table T with 8 row-replicas per email.  Edges are grouped host-side
into rounds so every scatter instruction has unique destination rows
(required: the DMA compute-op read-modify-write does not accumulate
duplicate indices within one instruction).  The dense phase streams
x_email.T in bf16, accumulates x@Wer into PSUM (spilled to SBUF so it
overlaps the scatter phase), then adds the T@Wcomb term after T is read
back transposed, applies relu and the tiny classifier.  No collectives.
"""
import numpy as np
from contextlib import ExitStack
import ml_dtypes

import concourse.bacc as bacc
import concourse.mybir as mybir
from concourse.bass import IndirectOffsetOnAxis
from concourse.bass_utils import run_bass_kernel_spmd

F32 = mybir.dt.float32
BF16 = mybir.dt.bfloat16
FP8 = mybir.dt.float8e4
F8 = ml_dtypes.float8_e4m3fn
I32 = mybir.dt.int32
BF = ml_dtypes.bfloat16

N_EMAIL, N_URL, N_SENDER = 100000, 400000, 50000
NCORE = 8
EPC = 12500                  # emails per core
EPAD = 12800                 # padded (25 blocks of 512)
NBLK, BW = 25, 512
R = 8                        # scatter row replicas
CAPS_U = [788, 592, 120, 8, 2, 1]   # ue group col caps (measured max +slack)
CAPS_S = [684, 120, 4, 1]           # se group col caps
GU = sum(CAPS_U)             # 1600
GS = sum(CAPS_S)             # 950
NG_U, NG_S = len(CAPS_U), len(CAPS_S)
NG = NG_U + NG_S
UBUF = CAPS_U[0]             # ring slot width (cols) for ue gather buf
SEBUF = CAPS_S[0]
TROWS = EPAD + 16            # 12816 rows of 96 (= EPAD*R + dump rows of 12)
DUMP = EPAD * R              # scatter dump row index (flat [TROWS*8, 12])
XRING = 6                    # x block ring depth
H1B = 12                     # blocks in T half 1
H1C = H1B * BW               # 6144

_prog_cache = {}


def _build_program():
    if "nc" in _prog_cache:
        return _prog_cache["nc"]
    nc = bacc.Bacc("TRN2")

    xT = nc.dram_tensor("xT", (768, EPAD), BF16, kind="ExternalInput")
    url_tab = nc.dram_tensor("url_tab", (N_URL + 1, 8), FP8, kind="ExternalInput")
    snd_tab = nc.dram_tensor("snd_tab", (N_SENDER + 1, 1), BF16, kind="ExternalInput")
    gidx_u = nc.dram_tensor("gidx_u", (128, GU), I32, kind="ExternalInput")
    sidx_u = nc.dram_tensor("sidx_u", (128, GU), I32, kind="ExternalInput")
    gidx_s = nc.dram_tensor("gidx_s", (128, GS), I32, kind="ExternalInput")
    sidx_s = nc.dram_tensor("sidx_s", (128, GS), I32, kind="ExternalInput")
    T = nc.dram_tensor("T", (TROWS, 96), BF16, kind="ExternalInput")
    wer = nc.dram_tensor("wer", (768, 128), BF16, kind="ExternalInput")
    wcomb = nc.dram_tensor("wcomb", (96, 128), BF16, kind="ExternalInput")
    wc = nc.dram_tensor("wc", (128, 2), BF16, kind="ExternalInput")
    ident = nc.dram_tensor("ident", (128, 128), BF16, kind="ExternalInput")
    out = nc.dram_tensor("out", (128, 8 * NBLK), F32, kind="ExternalOutput")

    with ExitStack() as ctx:
        E = ctx.enter_context
        gu_sb = E(nc.sbuf_tensor("gu_sb", (128, 2 * 8 * UBUF), FP8))
        gs_sb = E(nc.sbuf_tensor("gs_sb", (128, 2 * SEBUF), BF16))
        giu_sb = E(nc.sbuf_tensor("giu_sb", (128, GU), I32))
        siu_sb = E(nc.sbuf_tensor("siu_sb", (128, GU), I32))
        gis_sb = E(nc.sbuf_tensor("gis_sb", (128, GS), I32))
        sis_sb = E(nc.sbuf_tensor("sis_sb", (128, GS), I32))
        x_sb = E(nc.sbuf_tensor("x_sb", (128, (XRING + 4) * 6 * BW), BF16))
        tab_sb = E(nc.sbuf_tensor("tab_sb", (96, EPAD), BF16))
        zx_sb = E(nc.sbuf_tensor("zx_sb", (128, EPAD), BF16))
        zr_sb = E(nc.sbuf_tensor("zr_sb", (128, 4 * BW), BF16))
        wer_sb = E(nc.sbuf_tensor("wer_sb", (128, 768), BF16))
        wcomb_sb = E(nc.sbuf_tensor("wcomb_sb", (96, 128), BF16))
        wc_sb = E(nc.sbuf_tensor("wc_sb", (128, 2), BF16))
        id_sb = E(nc.sbuf_tensor("id_sb", (128, 128), BF16))
        out_sb = E(nc.sbuf_tensor("out_sb", (128, 8 * NBLK), F32))

        ps_z = [E(nc.psum_tensor(f"ps_z{i}", (128, BW), F32)) for i in range(2)]
        ps_z2 = [E(nc.psum_tensor(f"ps_z2{i}", (128, BW), F32)) for i in range(4)]
        ps_o = [E(nc.psum_tensor(f"ps_o{i}", (128, 8), F32)) for i in range(2)]

        # group metadata shared by host prep and device program
        caps = CAPS_U + CAPS_S
        offs_u = np.concatenate([[0], np.cumsum(CAPS_U)]).tolist()
        offs_s = np.concatenate([[0], np.cumsum(CAPS_S)]).tolist()

        sem_names = ["gusem", "susem", "gssem", "sssem", "wsem", "wersem", "gsA", "gsB",
                     "ssA", "ssB", "xs0", "xs1", "xs2", "xs3", "xs4", "xs5", "xpA", "xpB", "xpC", "xpD",
                     "tq0", "tq1", "tq2", "tq3", "zsem", "zxsem", "z2sem", "rsemA", "rsemB",
                     "osem", "ocsem", "odsem"]
        sems = {n: E(nc.semaphore(n)) for n in sem_names}
        (gusem, susem, gssem, sssem, wsem, wersem, gsA, gsB, ssA, ssB,
         xs0, xs1, xs2, xs3, xs4, xs5, xpA, xpB, xpC, xpD, tq0, tq1, tq2, tq3, zsem, zxsem, z2sem,
         rsemA, rsemB, osem, ocsem, odsem) = (sems[n] for n in sem_names)
        xsems = [xs0, xs1, xs2, xs3, xs4, xs5]
        xpsems = [xpA, xpB, xpC, xpD]
        tqsems = [tq0, tq1, tq2, tq3]
        ssems = [ssA, ssB]
        POOL_X = [18, 19, 24]               # x blocks loaded by Pool
        SP_X = [0, 2, 4, 6, 8, 10, 12, 14, 16, 20, 21, 22, 23]
        ACT_X = [1, 3, 5, 7, 9, 11, 13, 15, 17]
        main_list = [b for b in range(NBLK) if b not in POOL_X]
        slot_of, pos_of, prev_of = {}, {}, {}
        slot_hist = {}
        for idx, b in enumerate(main_list):
            s = idx % XRING
            slot_of[b] = s
            pos_of[b] = idx // XRING + 1
            prev_of[b] = slot_hist.get(s)
            slot_hist[s] = b
        pslot_hist = {}
        for idx, b in enumerate(POOL_X):
            s = XRING + idx % 4
            slot_of[b] = s
            pos_of[b] = idx // 4 + 1
            prev_of[b] = pslot_hist.get(s)
            pslot_hist[s] = b
        def xsem_of(b):
            return (xpsems[(slot_of[b] - XRING)] if b in POOL_X
                    else xsems[slot_of[b]])
        # T quarter boundaries: blocks [0:7), [7:13), [13:19), [19:25)
        TQB = [0, 7, 13, 19, NBLK]

        with nc.Block() as block:

            def x_block_dma(e, b):
                slot = slot_of[b]
                sem = xsem_of(b)
                if prev_of[b] is not None:
                    e.wait_ge(zsem, prev_of[b] + 1)
                e.dma_start(
                    out=x_sb[:, slot * 6 * BW:
                             (slot + 1) * 6 * BW].rearrange(
                        "p (k w) -> p k w", k=6),
                    in_=xT[:, b * BW:(b + 1) * BW].rearrange(
                        "(k p) w -> p k w", k=6),
                ).then_inc(sem, 16)

            @block.sync
            def _(sy):
                sy.dma_start(
                    out=wer_sb[:].rearrange("p (k h) -> p k h", k=6),
                    in_=wer[:].rearrange("(k p) h -> p k h", k=6),
                ).then_inc(wersem, 16)
                for b in SP_X:
                    x_block_dma(sy, b)
                    if b == 2:
                        sy.dma_start(out=siu_sb[:], in_=sidx_u[:]).then_inc(susem, 16)
                    if b == 16:
                        sy.dma_start(out=wcomb_sb[:], in_=wcomb[:]).then_inc(wsem, 16)
                        sy.dma_start(out=wc_sb[:], in_=wc[:]).then_inc(wsem, 16)
                        sy.dma_start(out=id_sb[:], in_=ident[:]).then_inc(wsem, 16)
                sy.wait_ge(ssA, 16 * ((NG + 1) // 2))
                sy.wait_ge(ssB, 16 * (NG // 2))
                with nc.allow_non_contiguous_dma(reason="transposed T load"):
                    for q in (2, 3):
                        c0, c1 = TQB[q] * BW, TQB[q + 1] * BW
                        sy.dma_start(
                            out=tab_sb[:, c0:c1],
                            in_=T[c0:c1, :].rearrange("d c -> c d"),
                        ).then_inc(tqsems[q], 16)
                sy.wait_ge(ocsem, 13)
                sy.dma_start(out=out[:, :104], in_=out_sb[:, :104]).then_inc(odsem, 16)
                sy.wait_ge(ocsem, NBLK)
                sy.dma_start(out=out[:, 104:], in_=out_sb[:, 104:]).then_inc(odsem, 16)
                sy.wait_ge(odsem, 32)

            @block.scalar
            def _(act):
                act.dma_start(out=giu_sb[:], in_=gidx_u[:]).then_inc(gusem, 16)
                for b in ACT_X:
                    x_block_dma(act, b)
                    if b == 5:
                        act.dma_start(out=gis_sb[:], in_=gidx_s[:]).then_inc(gssem, 16)
                    if b == 9:
                        act.dma_start(out=sis_sb[:], in_=sidx_s[:]).then_inc(sssem, 16)
                act.wait_ge(ssA, 16 * ((NG + 1) // 2))
                act.wait_ge(ssB, 16 * (NG // 2))
                with nc.allow_non_contiguous_dma(reason="transposed T load"):
                    for q in (0, 1):
                        c0, c1 = TQB[q] * BW, TQB[q + 1] * BW
                        act.dma_start(
                            out=tab_sb[:, c0:c1],
                            in_=T[c0:c1, :].rearrange("d c -> c d"),
                        ).then_inc(tqsems[q], 16)
                # relu for even blocks
                for b in range(0, NBLK, 2):
                    act.wait_ge(z2sem, b + 1)
                    if b >= 4:
                        act.wait_ge(osem, b - 3)
                    act.activation(
                        zr_sb[:, (b % 4) * BW:(b % 4 + 1) * BW],
                        ps_z2[b % 4][:],
                        mybir.ActivationFunctionType.Relu,
                    ).then_inc(rsemA, 1)

            def gather(gp, i):
                sem = gsA if i % 2 == 0 else gsB
                if i >= 2:
                    gp.wait_ge(ssems[i % 2], 16 * ((i - 2) // 2 + 1))
                if i < NG_U:
                    g = i
                    if g == 0:
                        gp.wait_ge(gusem, 16)
                    k = CAPS_U[g]
                    sl = (g % 2) * 8 * UBUF
                    gp.indirect_dma_start(
                        out=gu_sb[:, sl:sl + 8 * k],
                        out_offset=None,
                        in_=url_tab[:],
                        in_offset=IndirectOffsetOnAxis(
                            ap=giu_sb[:, offs_u[g]:offs_u[g] + k], axis=0),
                    ).then_inc(sem, 16)
                else:
                    g = i - NG_U
                    if g == 0:
                        gp.wait_ge(gssem, 16)
                    k = CAPS_S[g]
                    sl = (g % 2) * SEBUF
                    gp.indirect_dma_start(
                        out=gs_sb[:, sl:sl + k],
                        out_offset=None,
                        in_=snd_tab[:],
                        in_offset=IndirectOffsetOnAxis(
                            ap=gis_sb[:, offs_s[g]:offs_s[g] + k], axis=0),
                    ).then_inc(sem, 16)

            def scatter(gp, i, t_flat):
                sem = gsA if i % 2 == 0 else gsB
                gp.wait_ge(sem, 16 * (i // 2 + 1))
                if i < NG_U:
                    g = i
                    if g == 0:
                        gp.wait_ge(susem, 16)
                    k = CAPS_U[g]
                    sl = (g % 2) * 8 * UBUF
                    gp.indirect_dma_start(
                        out=t_flat,
                        out_offset=IndirectOffsetOnAxis(
                            ap=siu_sb[:, offs_u[g]:offs_u[g] + k], axis=0),
                        in_=gu_sb[:, sl:sl + 8 * k],
                        in_offset=None,
                        compute_op=mybir.AluOpType.add,
                    ).then_inc(ssems[i % 2], 16)
                else:
                    g = i - NG_U
                    if g == 0:
                        gp.wait_ge(sssem, 16)
                    k = CAPS_S[g]
                    sl = (g % 2) * SEBUF
                    gp.indirect_dma_start(
                        out=t_flat,
                        out_offset=IndirectOffsetOnAxis(
                            ap=sis_sb[:, offs_s[g]:offs_s[g] + k], axis=0),
                        in_=gs_sb[:, sl:sl + k],
                        in_offset=None,
                        compute_op=mybir.AluOpType.add,
                        element_offset=9,
                    ).then_inc(ssems[i % 2], 16)

            @block.gpsimd
            def _(gp):
                t_flat = T[:].rearrange("d (r c) -> (d r) c", r=R, c=12)
                gather(gp, 0)
                gather(gp, 1)
                for i in range(NG):
                    scatter(gp, i, t_flat)
                    if i + 2 < NG:
                        gather(gp, i + 2)
                for b in POOL_X:
                    x_block_dma(gp, b)

            def classifier(te, b):
                if b % 2 == 0:
                    te.wait_ge(rsemA, b // 2 + 1)
                else:
                    te.wait_ge(rsemB, (b - 1) // 2 + 1)
                if b >= 2:
                    te.wait_ge(ocsem, b - 1)
                for j in range(4):
                    inst = te.matmul(
                        ps_o[b % 2][:, 2 * j:2 * j + 2],
                        zr_sb[:, (b % 4) * BW + j * 128:
                              (b % 4) * BW + (j + 1) * 128],
                        wc_sb[:],
                        start=True,
                        stop=True,
                    )
                    if j == 3:
                        inst.then_inc(osem, 1)

            @block.tensor
            def _(te):
                te.wait_ge(wersem, 16)
                # phase a: x @ Wer accumulated per block, spilled by DVE
                for b in range(NBLK):
                    te.wait_ge(xsem_of(b), 16 * pos_of[b])
                    base = slot_of[b] * 6 * BW
                    if b >= 2:
                        te.wait_ge(zxsem, b - 1)
                    for k in range(6):
                        inst = te.matmul(
                            ps_z[b % 2][:],
                            wer_sb[:, k * 128:(k + 1) * 128],
                            x_sb[:, base + k * BW:base + (k + 1) * BW],
                            start=(k == 0),
                            stop=(k == 5),
                        )
                        if k == 5:
                            inst.then_inc(zsem, 1)
                # phase b: + zx (identity) + T @ Wcomb, then relu + classifier
                te.wait_ge(wsem, 48)
                for b in range(NBLK):
                    te.wait_ge(zxsem, b + 1)
                    q = next(i for i in range(4) if b < TQB[i + 1])
                    te.wait_ge(tqsems[q], 16)
                    if b >= 4:
                        if (b - 4) % 2 == 0:
                            te.wait_ge(rsemA, (b - 4) // 2 + 1)
                        else:
                            te.wait_ge(rsemB, (b - 3) // 2)
                    te.matmul(
                        ps_z2[b % 4][:],
                        id_sb[:],
                        zx_sb[:, b * BW:(b + 1) * BW],
                        start=True,
                        stop=False,
                    )
                    te.matmul(
                        ps_z2[b % 4][:],
                        wcomb_sb[:],
                        tab_sb[:, b * BW:(b + 1) * BW],
                        start=False,
                        stop=True,
                    ).then_inc(z2sem, 1)
                    if b >= 4:
                        classifier(te, b - 4)
                classifier(te, NBLK - 4)
                classifier(te, NBLK - 3)
                classifier(te, NBLK - 2)
                classifier(te, NBLK - 1)

            @block.vector
            def _(ve):
                for b in range(NBLK):
                    ve.wait_ge(zsem, b + 1)
                    ve.tensor_copy(
                        out=zx_sb[:, b * BW:(b + 1) * BW],
                        in_=ps_z[b % 2][:],
                    ).then_inc(zxsem, 1)
                for b in range(NBLK):
                    if b % 2 == 1:
                        ve.wait_ge(z2sem, b + 1)
                        if b >= 4:
                            ve.wait_ge(osem, b - 3)
                        ve.tensor_scalar_max(
                            zr_sb[:, (b % 4) * BW:(b % 4 + 1) * BW],
                            ps_z2[b % 4][:],
                            0.0,
                        ).then_inc(rsemB, 1)
                    if b >= 1:
                        ve.wait_ge(osem, b)
                        ve.tensor_copy(
                            out=out_sb[:, (b - 1) * 8:b * 8],
                            in_=ps_o[(b - 1) % 2][:],
                        ).then_inc(ocsem, 1)
                ve.wait_ge(osem, NBLK)
                ve.tensor_copy(
                    out=out_sb[:, (NBLK - 1) * 8:NBLK * 8],
                    in_=ps_o[(NBLK - 1) % 2][:],
                ).then_inc(ocsem, 1)

    nc.compile()
    _prog_cache["nc"] = nc
    return nc


def _pack_groups(src, dst, caps, gpad, spad):
    """Group edges into rounds of R so each group has unique (dst, rep)."""
    order = np.argsort(dst, kind="stable")
    ds, ss = dst[order], src[order]
    starts = np.searchsorted(ds, np.arange(EPAD + 1))
    ranks = np.arange(len(ds)) - starts[ds]
    rep = ranks % R
    grp = ranks // R
    ncols = sum(caps)
    gidx = np.full((128, ncols), gpad, np.int32)
    sidx = np.full((128, ncols), spad, np.int32)
    if len(ds) and grp.max() >= len(caps):
        raise AssertionError(f"degree overflow: max grp {grp.max()}")
    off = 0
    for g, cap in enumerate(caps):
        m = grp == g
        n = int(m.sum())
        assert n <= 128 * cap, f"group {g} overflow: {n} > {128 * cap}"
        bg = np.full(128 * cap, gpad, np.int32)
        bs = np.full(128 * cap, spad, np.int32)
        bg[:n] = ss[m]
        bs[:n] = ds[m] * R + rep[m]
        gidx[:, off:off + cap] = bg.reshape(128, cap)
        sidx[:, off:off + cap] = bs.reshape(128, cap)
        off += cap
    return gidx, sidx


def _host_prep(inputs):
    f32 = np.float32
    x_email = np.asarray(inputs["x_email"], f32)
    x_url = np.asarray(inputs["x_url"], f32)
    x_sender = np.asarray(inputs["x_sender"], f32)

    url_tab = np.zeros((N_URL + 1, 8), F8)
    url_tab[:N_URL] = x_url.astype(F8)
    snd_tab = np.zeros((N_SENDER + 1, 1), BF)
    snd_tab[:N_SENDER, 0] = x_sender[:, 0].astype(BF)

    wroot = inputs["Wroot_ue"] + inputs["Wroot_se"]
    wer = np.ascontiguousarray((inputs["W_email"] @ wroot)).astype(BF)
    wcomb12 = np.zeros((12, 128), f32)
    wcomb12[0:8] = inputs["W_url"] @ inputs["Wrel_ue"]
    wcomb12[8] = inputs["b_url"] @ inputs["Wrel_ue"]
    wcomb12[9] = inputs["W_sender"][0] @ inputs["Wrel_se"]
    wcomb12[10] = inputs["b_sender"] @ inputs["Wrel_se"]
    wcomb12[11] = (inputs["brel_ue"] + inputs["brel_se"]
                   + inputs["b_email"] @ wroot)
    wcomb96 = np.tile(wcomb12, (R, 1)).astype(BF)
    wc = np.ascontiguousarray(inputs["Wc"]).astype(BF)
    ident = np.eye(128, dtype=BF)

    src_ue = np.asarray(inputs["src_ue"], np.int32)
    dst_ue = np.asarray(inputs["dst_ue"], np.int32)
    src_se = np.asarray(inputs["src_se"], np.int32)
    dst_se = np.asarray(inputs["dst_se"], np.int32)

    in_maps = []
    for c in range(NCORE):
        lo, hi = c * EPC, (c + 1) * EPC
        mu = (dst_ue >= lo) & (dst_ue < hi)
        su, du = src_ue[mu], dst_ue[mu] - lo
        ms = (dst_se >= lo) & (dst_se < hi)
        ss_, ds_ = src_se[ms], dst_se[ms] - lo

        gixu, sixu = _pack_groups(su, du, CAPS_U, N_URL, DUMP)
        gixs, sixs = _pack_groups(ss_, ds_, CAPS_S, N_SENDER, DUMP)

        T0 = np.zeros((TROWS, 96), BF)
        deg_u = np.bincount(du, minlength=EPAD).astype(f32)
        deg_s = np.bincount(ds_, minlength=EPAD).astype(f32)
        T0[:EPAD, 8] = deg_u.astype(BF)
        T0[:EPAD, 10] = deg_s.astype(BF)
        T0[:EPAD, 11] = 1.0

        xTc = np.zeros((768, EPAD), BF)
        xTc[:, :EPC] = x_email[lo:hi].T.astype(BF)

        in_maps.append({
            "xT": xTc,
            "url_tab": url_tab,
            "snd_tab": snd_tab,
            "gidx_u": gixu,
            "sidx_u": sixu,
            "gidx_s": gixs,
            "sidx_s": sixs,
            "T": T0,
            "wer": wer,
            "wcomb": wcomb96,
            "wc": wc,
            "ident": ident,
        })
    return in_maps


def kernel(**inputs):
    nc = _build_program()
    in_maps = _host_prep(inputs)
    res = None
    last_exc = None
    for _attempt in range(3):
        try:
            res = run_bass_kernel_spmd(nc, in_maps, list(range(NCORE)))
            break
        except Exception as e:  # transient device wedge recovers on retry
            last_exc = e
            import time as _time
            _time.sleep(5.0)
    if res is None:
        raise last_exc
    out = np.empty((N_EMAIL, 2), np.float32)
    bc = np.asarray(inputs["bc"], np.float32)
    for c in range(NCORE):
        r = np.asarray(res.results[c]["out"])  # [128, 200]
        full = r.reshape(128, NBLK, 4, 2).transpose(1, 2, 0, 3).reshape(EPAD, 2)
        out[c * EPC:(c + 1) * EPC] = full[:EPC]
    return out + bc


# revision 20
# speedup vs baseline: 1.0899x; 1.0899x over previous
"""Trainium2 Bass kernel for HGNN-MLP (email/url/sender heterograph).

Math (dead-code-eliminated: out_url/out_sender unused by the return value):
  out = relu( x_email @ Wer + T @ Wcomb )[:, :] @ Wc + bc
where
  Wer  = W_email @ (Wroot_ue + Wroot_se)                      [768,128]
  T[d] = [sum x_url[src] over ue edges, deg_ue, sum x_sender[src]
          over se edges, deg_se, 1]  (12 cols, 8 replicas)
  Wcomb folds W_url@Wrel_ue, b_url@Wrel_ue, W_sender@Wrel_se,
          b_sender@Wrel_se and the bias row.

Distribution: 8-way data-parallel over destination emails (12500/core),
edge lists bucketed by dst partition on host; small weights replicated.

Device strategy per core: batched indirect-DMA gathers of source rows
(url: 8 bf16, sender: 1 bf16) followed by indirect-DMA scatter-ADD into a
DRAM table T with 8 row-replicas per email.  Edges are grouped host-side
into rounds so every scatter instruction has unique destination rows
(required: the DMA compute-op read-modify-write does not accumulate
duplicate indices within one instruction).  The dense phase streams
x_email.T in bf16, accumulates x@Wer into PSUM (spilled to SBUF so it
overlaps the scatter phase), then adds the T@Wcomb term after T is read
back transposed, applies relu and the tiny classifier.  No collectives.
"""
import numpy as np
from contextlib import ExitStack
import ml_dtypes

import concourse.bacc as bacc
import concourse.mybir as mybir
from concourse.bass import IndirectOffsetOnAxis
from concourse.bass_utils import run_bass_kernel_spmd

F32 = mybir.dt.float32
BF16 = mybir.dt.bfloat16
FP8 = mybir.dt.float8e4
F8 = ml_dtypes.float8_e4m3fn
I32 = mybir.dt.int32
BF = ml_dtypes.bfloat16

N_EMAIL, N_URL, N_SENDER = 100000, 400000, 50000
NCORE = 8
EPC = 12500                  # emails per core
EPAD = 12800                 # padded (25 blocks of 512)
NBLK, BW = 25, 512
R = 8                        # scatter row replicas
CAPS_U = [788, 592, 120, 8, 2, 1]   # ue group col caps (measured max +slack)
CAPS_S = [684, 120, 4, 1]           # se group col caps
GU = sum(CAPS_U)             # 1600
GS = sum(CAPS_S)             # 950
NG_U, NG_S = len(CAPS_U), len(CAPS_S)
NG = NG_U + NG_S
UBUF = CAPS_U[0]             # ring slot width (cols) for ue gather buf
SEBUF = CAPS_S[0]
TROWS = EPAD + 16            # 12816 rows of 96 (= EPAD*R + dump rows of 12)
DUMP = EPAD * R              # scatter dump row index (flat [TROWS*8, 12])
XRING = 6                    # x block ring depth
H1B = 12                     # blocks in T half 1
H1C = H1B * BW               # 6144

_prog_cache = {}


def _build_program():
    if "nc" in _prog_cache:
        return _prog_cache["nc"]
    nc = bacc.Bacc("TRN2")

    xT = nc.dram_tensor("xT", (768, EPAD), BF16, kind="ExternalInput")
    url_tab = nc.dram_tensor("url_tab", (N_URL + 1, 8), FP8, kind="ExternalInput")
    snd_tab = nc.dram_tensor("snd_tab", (N_SENDER + 1, 1), BF16, kind="ExternalInput")
    gidx_u = nc.dram_tensor("gidx_u", (128, GU), I32, kind="ExternalInput")
    sidx_u = nc.dram_tensor("sidx_u", (128, GU), I32, kind="ExternalInput")
    gidx_s = nc.dram_tensor("gidx_s", (128, GS), I32, kind="ExternalInput")
    sidx_s = nc.dram_tensor("sidx_s", (128, GS), I32, kind="ExternalInput")
    T = nc.dram_tensor("T", (TROWS, 96), BF16, kind="ExternalInput")
    wer = nc.dram_tensor("wer", (768, 128), BF16, kind="ExternalInput")
    wcomb = nc.dram_tensor("wcomb", (96, 128), BF16, kind="ExternalInput")
    wc = nc.dram_tensor("wc", (128, 2), BF16, kind="ExternalInput")
    ident = nc.dram_tensor("ident", (128, 128), BF16, kind="ExternalInput")
    out = nc.dram_tensor("out", (128, 8 * NBLK), F32, kind="ExternalOutput")

    with ExitStack() as ctx:
        E = ctx.enter_context
        gu_sb = E(nc.sbuf_tensor("gu_sb", (128, 2 * 8 * UBUF), FP8))
        gs_sb = E(nc.sbuf_tensor("gs_sb", (128, 2 * SEBUF), BF16))
        giu_sb = E(nc.sbuf_tensor("giu_sb", (128, GU), I32))
        siu_sb = E(nc.sbuf_tensor("siu_sb", (128, GU), I32))
        gis_sb = E(nc.sbuf_tensor("gis_sb", (128, GS), I32))
        sis_sb = E(nc.sbuf_tensor("sis_sb", (128, GS), I32))
        x_sb = E(nc.sbuf_tensor("x_sb", (128, (XRING + 4) * 6 * BW), BF16))
        tab_sb = E(nc.sbuf_tensor("tab_sb", (96, EPAD), BF16))
        zx_sb = E(nc.sbuf_tensor("zx_sb", (128, EPAD), BF16))
        zr_sb = E(nc.sbuf_tensor("zr_sb", (128, 4 * BW), BF16))
        wer_sb = E(nc.sbuf_tensor("wer_sb", (128, 768), BF16))
        wcomb_sb = E(nc.sbuf_tensor("wcomb_sb", (96, 128), BF16))
        wc_sb = E(nc.sbuf_tensor("wc_sb", (128, 2), BF16))
        id_sb = E(nc.sbuf_tensor("id_sb", (128, 128), BF16))
        out_sb = E(nc.sbuf_tensor("out_sb", (128, 8 * NBLK), F32))

        ps_z = [E(nc.psum_tensor(f"ps_z{i}", (128, BW), F32)) for i in range(2)]
        ps_z2 = [E(nc.psum_tensor(f"ps_z2{i}", (128, BW), F32)) for i in range(4)]
        ps_o = [E(nc.psum_tensor(f"ps_o{i}", (128, 8), F32)) for i in range(2)]

        # group metadata shared by host prep and device program
        caps = CAPS_U + CAPS_S
        offs_u = np.concatenate([[0], np.cumsum(CAPS_U)]).tolist()
        offs_s = np.concatenate([[0], np.cumsum(CAPS_S)]).tolist()

        sem_names = ["gusem", "susem", "gssem", "sssem", "wsem", "wersem", "gsA", "gsB",
                     "ssA", "ssB", "xs0", "xs1", "xs2", "xs3", "xs4", "xs5", "xpA", "xpB", "xpC", "xpD",
                     "tq0", "tq1", "tq2", "tq3", "zsem", "zxsem", "z2sem", "rsemA", "rsemB",
                     "osem", "ocsem", "odsem"]
        sems = {n: E(nc.semaphore(n)) for n in sem_names}
        (gusem, susem, gssem, sssem, wsem, wersem, gsA, gsB, ssA, ssB,
         xs0, xs1, xs2, xs3, xs4, xs5, xpA, xpB, xpC, xpD, tq0, tq1, tq2, tq3, zsem, zxsem, z2sem,
         rsemA, rsemB, osem, ocsem, odsem) = (sems[n] for n in sem_names)
        xsems = [xs0, xs1, xs2, xs3, xs4, xs5]
        xpsems = [xpA, xpB, xpC, xpD]
        tqsems = [tq0, tq1, tq2, tq3]
        ssems = [ssA, ssB]
        POOL_X = [18, 19, 24]               # x blocks loaded by Pool
        SP_X = [0, 2, 4, 6, 8, 10, 12, 14, 16, 20, 21, 22, 23]
        ACT_X = [1, 3, 5, 7, 9, 11, 13, 15, 17]
        main_list = [b for b in range(NBLK) if b not in POOL_X]
        slot_of, pos_of, prev_of = {}, {}, {}
        slot_hist = {}
        for idx, b in enumerate(main_list):
            s = idx % XRING
            slot_of[b] = s
            pos_of[b] = idx // XRING + 1
            prev_of[b] = slot_hist.get(s)
            slot_hist[s] = b
        pslot_hist = {}
        for idx, b in enumerate(POOL_X):
            s = XRING + idx % 4
            slot_of[b] = s
            pos_of[b] = idx // 4 + 1
            prev_of[b] = pslot_hist.get(s)
            pslot_hist[s] = b
        def xsem_of(b):
            return (xpsems[(slot_of[b] - XRING)] if b in POOL_X
                    else xsems[slot_of[b]])
        # T quarter boundaries: blocks [0:7), [7:13), [13:19), [19:25)
        TQB = [0, 7, 13, 19, NBLK]

        with nc.Block() as block:

            def x_block_dma(e, b):
                slot = slot_of[b]
                sem = xsem_of(b)
                if prev_of[b] is not None:
                    e.wait_ge(zsem, prev_of[b] + 1)
                e.dma_start(
                    out=x_sb[:, slot * 6 * BW:
                             (slot + 1) * 6 * BW].rearrange(
                        "p (k w) -> p k w", k=6),
                    in_=xT[:, b * BW:(b + 1) * BW].rearrange(
                        "(k p) w -> p k w", k=6),
                ).then_inc(sem, 16)

            @block.sync
            def _(sy):
                sy.dma_start(
                    out=wer_sb[:].rearrange("p (k h) -> p k h", k=6),
                    in_=wer[:].rearrange("(k p) h -> p k h", k=6),
                ).then_inc(wersem, 16)
                for b in SP_X:
                    x_block_dma(sy, b)
                    if b == 2:
                        sy.dma_start(out=siu_sb[:], in_=sidx_u[:]).then_inc(susem, 16)
                    if b == 16:
                        sy.dma_start(out=wcomb_sb[:], in_=wcomb[:]).then_inc(wsem, 16)
                        sy.dma_start(out=wc_sb[:], in_=wc[:]).then_inc(wsem, 16)
                        sy.dma_start(out=id_sb[:], in_=ident[:]).then_inc(wsem, 16)
                sy.wait_ge(ssA, 16 * ((NG + 1) // 2))
                sy.wait_ge(ssB, 16 * (NG // 2))
                with nc.allow_non_contiguous_dma(reason="transposed T load"):
                    for q in (2, 3):
                        c0, c1 = TQB[q] * BW, TQB[q + 1] * BW
                        sy.dma_start(
                            out=tab_sb[:, c0:c1],
                            in_=T[c0:c1, :].rearrange("d c -> c d"),
                        ).then_inc(tqsems[q], 16)
                sy.wait_ge(ocsem, 13)
                sy.dma_start(out=out[:, :104], in_=out_sb[:, :104]).then_inc(odsem, 16)
                sy.wait_ge(ocsem, NBLK)
                sy.dma_start(out=out[:, 104:], in_=out_sb[:, 104:]).then_inc(odsem, 16)
                sy.wait_ge(odsem, 32)

            @block.scalar
            def _(act):
                act.dma_start(out=giu_sb[:], in_=gidx_u[:]).then_inc(gusem, 16)
                for b in ACT_X:
                    x_block_dma(act, b)
                    if b == 5:
                        act.dma_start(out=gis_sb[:], in_=gidx_s[:]).then_inc(gssem, 16)
                    if b == 9:
                        act.dma_start(out=sis_sb[:], in_=sidx_s[:]).then_inc(sssem, 16)
                act.wait_ge(ssA, 16 * ((NG + 1) // 2))
                act.wait_ge(ssB, 16 * (NG // 2))
                with nc.allow_non_contiguous_dma(reason="transposed T load"):
                    for q in (0, 1):
                        c0, c1 = TQB[q] * BW, TQB[q + 1] * BW
                        act.dma_start(
                            out=tab_sb[:, c0:c1],
                            in_=T[c0:c1, :].rearrange("d c -> c d"),
                        ).then_inc(tqsems[q], 16)
                # relu for even blocks
                for b in range(0, NBLK, 2):
                    act.wait_ge(z2sem, b + 1)
                    if b >= 4:
                        act.wait_ge(osem, b - 3)
                    act.activation(
                        zr_sb[:, (b % 4) * BW:(b % 4 + 1) * BW],
                        ps_z2[b % 4][:],
                        mybir.ActivationFunctionType.Relu,
                    ).then_inc(rsemA, 1)

            def gather(gp, i):
                sem = gsA if i % 2 == 0 else gsB
                if i >= 2:
                    gp.wait_ge(ssems[i % 2], 16 * ((i - 2) // 2 + 1))
                if i < NG_U:
                    g = i
                    if g == 0:
                        gp.wait_ge(gusem, 16)
                    k = CAPS_U[g]
                    sl = (g % 2) * 8 * UBUF
                    gp.indirect_dma_start(
                        out=gu_sb[:, sl:sl + 8 * k],
                        out_offset=None,
                        in_=url_tab[:],
                        in_offset=IndirectOffsetOnAxis(
                            ap=giu_sb[:, offs_u[g]:offs_u[g] + k], axis=0),
                    ).then_inc(sem, 16)
                else:
                    g = i - NG_U
                    if g == 0:
                        gp.wait_ge(gssem, 16)
                    k = CAPS_S[g]
                    sl = (g % 2) * SEBUF
                    gp.indirect_dma_start(
                        out=gs_sb[:, sl:sl + k],
                        out_offset=None,
                        in_=snd_tab[:],
                        in_offset=IndirectOffsetOnAxis(
                            ap=gis_sb[:, offs_s[g]:offs_s[g] + k], axis=0),
                    ).then_inc(sem, 16)

            def scatter(gp, i, t_flat):
                sem = gsA if i % 2 == 0 else gsB
                gp.wait_ge(sem, 16 * (i // 2 + 1))
                if i < NG_U:
                    g = i
                    if g == 0:
                        gp.wait_ge(susem, 16)
                    k = CAPS_U[g]
                    sl = (g % 2) * 8 * UBUF
                    gp.indirect_dma_start(
                        out=t_flat,
                        out_offset=IndirectOffsetOnAxis(
                            ap=siu_sb[:, offs_u[g]:offs_u[g] + k], axis=0),
                        in_=gu_sb[:, sl:sl + 8 * k],
                        in_offset=None,
                        compute_op=mybir.AluOpType.add,
                    ).then_inc(ssems[i % 2], 16)
                else:
                    g = i - NG_U
                    if g == 0:
                        gp.wait_ge(sssem, 16)
                    k = CAPS_S[g]
                    sl = (g % 2) * SEBUF
                    gp.indirect_dma_start(
                        out=t_flat,
                        out_offset=IndirectOffsetOnAxis(
                            ap=sis_sb[:, offs_s[g]:offs_s[g] + k], axis=0),
                        in_=gs_sb[:, sl:sl + k],
                        in_offset=None,
                        compute_op=mybir.AluOpType.add,
                        element_offset=9,
                    ).then_inc(ssems[i % 2], 16)

            @block.gpsimd
            def _(gp):
                t_flat = T[:].rearrange("d (r c) -> (d r) c", r=R, c=12)
                gather(gp, 0)
                gather(gp, 1)
                for i in range(NG):
                    scatter(gp, i, t_flat)
                    if i + 2 < NG:
                        gather(gp, i + 2)
                for b in POOL_X:
                    x_block_dma(gp, b)

            def classifier(te, b):
                if b % 2 == 0:
                    te.wait_ge(rsemA, b // 2 + 1)
                else:
                    te.wait_ge(rsemB, (b - 1) // 2 + 1)
                if b >= 2:
                    te.wait_ge(ocsem, b - 1)
                for j in range(4):
                    inst = te.matmul(
                        ps_o[b % 2][:, 2 * j:2 * j + 2],
                        zr_sb[:, (b % 4) * BW + j * 128:
                              (b % 4) * BW + (j + 1) * 128],
                        wc_sb[:],
                        start=True,
                        stop=True,
                    )
                    if j == 3:
                        inst.then_inc(osem, 1)

            @block.tensor
            def _(te):
                te.wait_ge(wersem, 16)
                # phase a: x @ Wer accumulated per block, spilled by DVE
                for b in range(NBLK):
                    te.wait_ge(xsem_of(b), 16 * pos_of[b])
                    base = slot_of[b] * 6 * BW
                    if b >= 2:
                        te.wait_ge(zxsem, b - 1)
                    for k in range(6):
                        inst = te.matmul(
                            ps_z[b % 2][:],
                            wer_sb[:, k * 128:(k + 1) * 128],
                            x_sb[:, base + k * BW:base + (k + 1) * BW],
                            start=(k == 0),
                            stop=(k == 5),
                        )
                        if k == 5:
                            inst.then_inc(zsem, 1)
                # phase b: + zx (identity) + T @ Wcomb, then relu + classifier
                te.wait_ge(wsem, 48)
                for b in range(NBLK):
                    te.wait_ge(zxsem, b + 1)
                    q = next(i for i in range(4) if b < TQB[i + 1])
                    te.wait_ge(tqsems[q], 16)
                    if b >= 4:
                        if (b - 4) % 2 == 0:
                            te.wait_ge(rsemA, (b - 4) // 2 + 1)
                        else:
                            te.wait_ge(rsemB, (b - 3) // 2)
                    te.matmul(
                        ps_z2[b % 4][:],
                        id_sb[:],
                        zx_sb[:, b * BW:(b + 1) * BW],
                        start=True,
                        stop=False,
                    )
                    te.matmul(
                        ps_z2[b % 4][:],
                        wcomb_sb[:],
                        tab_sb[:, b * BW:(b + 1) * BW],
                        start=False,
                        stop=True,
                    ).then_inc(z2sem, 1)
                    if b >= 3:
                        classifier(te, b - 3)
                classifier(te, NBLK - 3)
                classifier(te, NBLK - 2)
                classifier(te, NBLK - 1)

            @block.vector
            def _(ve):
                for b in range(NBLK):
                    ve.wait_ge(zsem, b + 1)
                    ve.tensor_copy(
                        out=zx_sb[:, b * BW:(b + 1) * BW],
                        in_=ps_z[b % 2][:],
                    ).then_inc(zxsem, 1)
                for b in range(NBLK):
                    if b % 2 == 1:
                        ve.wait_ge(z2sem, b + 1)
                        if b >= 4:
                            ve.wait_ge(osem, b - 3)
                        ve.tensor_scalar_max(
                            zr_sb[:, (b % 4) * BW:(b % 4 + 1) * BW],
                            ps_z2[b % 4][:],
                            0.0,
                        ).then_inc(rsemB, 1)
                    if b >= 1:
                        ve.wait_ge(osem, b)
                        ve.tensor_copy(
                            out=out_sb[:, (b - 1) * 8:b * 8],
                            in_=ps_o[(b - 1) % 2][:],
                        ).then_inc(ocsem, 1)
                ve.wait_ge(osem, NBLK)
                ve.tensor_copy(
                    out=out_sb[:, (NBLK - 1) * 8:NBLK * 8],
                    in_=ps_o[(NBLK - 1) % 2][:],
                ).then_inc(ocsem, 1)

    nc.compile()
    _prog_cache["nc"] = nc
    return nc


def _pack_groups(src, dst, caps, gpad, spad):
    """Group edges into rounds of R so each group has unique (dst, rep)."""
    order = np.argsort(dst, kind="stable")
    ds, ss = dst[order], src[order]
    starts = np.searchsorted(ds, np.arange(EPAD + 1))
    ranks = np.arange(len(ds)) - starts[ds]
    rep = ranks % R
    grp = ranks // R
    ncols = sum(caps)
    gidx = np.full((128, ncols), gpad, np.int32)
    sidx = np.full((128, ncols), spad, np.int32)
    if len(ds) and grp.max() >= len(caps):
        raise AssertionError(f"degree overflow: max grp {grp.max()}")
    off = 0
    for g, cap in enumerate(caps):
        m = grp == g
        n = int(m.sum())
        assert n <= 128 * cap, f"group {g} overflow: {n} > {128 * cap}"
        bg = np.full(128 * cap, gpad, np.int32)
        bs = np.full(128 * cap, spad, np.int32)
        bg[:n] = ss[m]
        bs[:n] = ds[m] * R + rep[m]
        gidx[:, off:off + cap] = bg.reshape(128, cap)
        sidx[:, off:off + cap] = bs.reshape(128, cap)
        off += cap
    return gidx, sidx


def _host_prep(inputs):
    f32 = np.float32
    x_email = np.asarray(inputs["x_email"], f32)
    x_url = np.asarray(inputs["x_url"], f32)
    x_sender = np.asarray(inputs["x_sender"], f32)

    url_tab = np.zeros((N_URL + 1, 8), F8)
    url_tab[:N_URL] = x_url.astype(F8)
    snd_tab = np.zeros((N_SENDER + 1, 1), BF)
    snd_tab[:N_SENDER, 0] = x_sender[:, 0].astype(BF)

    wroot = inputs["Wroot_ue"] + inputs["Wroot_se"]
    wer = np.ascontiguousarray((inputs["W_email"] @ wroot)).astype(BF)
    wcomb12 = np.zeros((12, 128), f32)
    wcomb12[0:8] = inputs["W_url"] @ inputs["Wrel_ue"]
    wcomb12[8] = inputs["b_url"] @ inputs["Wrel_ue"]
    wcomb12[9] = inputs["W_sender"][0] @ inputs["Wrel_se"]
    wcomb12[10] = inputs["b_sender"] @ inputs["Wrel_se"]
    wcomb12[11] = (inputs["brel_ue"] + inputs["brel_se"]
                   + inputs["b_email"] @ wroot)
    wcomb96 = np.tile(wcomb12, (R, 1)).astype(BF)
    wc = np.ascontiguousarray(inputs["Wc"]).astype(BF)
    ident = np.eye(128, dtype=BF)

    src_ue = np.asarray(inputs["src_ue"], np.int32)
    dst_ue = np.asarray(inputs["dst_ue"], np.int32)
    src_se = np.asarray(inputs["src_se"], np.int32)
    dst_se = np.asarray(inputs["dst_se"], np.int32)

    in_maps = []
    for c in range(NCORE):
        lo, hi = c * EPC, (c + 1) * EPC
        mu = (dst_ue >= lo) & (dst_ue < hi)
        su, du = src_ue[mu], dst_ue[mu] - lo
        ms = (dst_se >= lo) & (dst_se < hi)
        ss_, ds_ = src_se[ms], dst_se[ms] - lo

        gixu, sixu = _pack_groups(su, du, CAPS_U, N_URL, DUMP)
        gixs, sixs = _pack_groups(ss_, ds_, CAPS_S, N_SENDER, DUMP)

        T0 = np.zeros((TROWS, 96), BF)
        deg_u = np.bincount(du, minlength=EPAD).astype(f32)
        deg_s = np.bincount(ds_, minlength=EPAD).astype(f32)
        T0[:EPAD, 8] = deg_u.astype(BF)
        T0[:EPAD, 10] = deg_s.astype(BF)
        T0[:EPAD, 11] = 1.0

        xTc = np.zeros((768, EPAD), BF)
        xTc[:, :EPC] = x_email[lo:hi].T.astype(BF)

        in_maps.append({
            "xT": xTc,
            "url_tab": url_tab,
            "snd_tab": snd_tab,
            "gidx_u": gixu,
            "sidx_u": sixu,
            "gidx_s": gixs,
            "sidx_s": sixs,
            "T": T0,
            "wer": wer,
            "wcomb": wcomb96,
            "wc": wc,
            "ident": ident,
        })
    return in_maps


def kernel(**inputs):
    nc = _build_program()
    in_maps = _host_prep(inputs)
    res = None
    last_exc = None
    for _attempt in range(3):
        try:
            res = run_bass_kernel_spmd(nc, in_maps, list(range(NCORE)))
            break
        except Exception as e:  # transient device wedge recovers on retry
            last_exc = e
            import time as _time
            _time.sleep(5.0)
    if res is None:
        raise last_exc
    out = np.empty((N_EMAIL, 2), np.float32)
    bc = np.asarray(inputs["bc"], np.float32)
    for c in range(NCORE):
        r = np.asarray(res.results[c]["out"])  # [128, 200]
        full = r.reshape(128, NBLK, 4, 2).transpose(1, 2, 0, 3).reshape(EPAD, 2)
        out[c * EPC:(c + 1) * EPC] = full[:EPC]
    return out + bc
